# revision 1
# baseline (speedup 1.0000x reference)
"""CRF negative-log-likelihood kernel for Trainium2, SPMD over 8 NeuronCores.

Strategy
--------
Data-parallel over batch: core c handles sequences b in [c*8, (c+1)*8).

Per core (B=8 local sequences, T=512, K=50 tags, D=1024), all fp32:

1. Emissions GEMM in transposed layout emisT[k, bt]:  TensorE contracts
   the partition dim, so the moving operand must be hidden^T.  hidden is
   HWDGE-loaded, transposed 128x128-wise on the TensorE (identity
   matmul, PSUM out), copied PSUM->SBUF by DVE, then the GEMM
   accumulates 8 d-chunks with W (already d-major in DRAM) stationary.
2. Partition function: linear-domain forward recurrence
       alpha_t = (expT^T @ alpha_{t-1}) * E_t
   with E = exp(emisT + b).  Two independent chains (sequences 0-3 on
   partitions 0-49, 4-7 on partitions 64-113 via PE row/col groups)
   overlap each other's PE<->DVE latency.  Every RENORM steps a rank-1
   matmul sums alpha; the reciprocal is broadcast back over partitions
   with another rank-1 matmul and folded into the E column two steps
   ahead (scale propagates linearly); log(sum) accumulates into C.
   log_Z = log(sum_j alpha_T) + C, with exp(end_trans) pre-folded into
   the last E column and exp(start_trans) into alpha_0.
3. Gold path score via one-hot algebra (no gathers):
       OH[k, (b,t)] = (k == tag[b,t])       (iota compare of a rank-1
                                             broadcast matmul of tags)
       R[k, (b,t)]  = trans[tag[b,t-1], k]  (matmul: trans^T @ OH shifted)
       G = emisT + b + R, G[:,b,0] += start, G[:,b,511] += end
       gold[b] = sum_{k,t} G * OH           (DVE mul+reduce, ones matmul)
4. out[b] = log_Z[b] - gold[b].
"""

import numpy as np

B_FULL = 64
B_LOC = 8
BH = 4  # sequences per chain
T = 512
K = 50
D = 1024
BT = B_LOC * T  # 4096
N_CORES = 8
D_CHUNKS = D // 128  # 8
RENORM = 8
H2 = 64  # partition base of chain B

_COMPILED = {}
LAST_RESULT = None


def _build(dbg=False):
    import concourse.bass as bass
    import concourse.tile as tile
    from concourse import bacc, mybir

    f32 = mybir.dt.float32

    nc = bacc.Bacc(
        "TRN2",
        target_bir_lowering=False,
        debug=False,
        num_devices=N_CORES,
    )

    hid = nc.dram_tensor("hid", [BT, D], f32, kind="ExternalInput")
    wq = nc.dram_tensor("wq", [D_CHUNKS, 128, K], f32, kind="ExternalInput")
    ident = nc.dram_tensor("ident", [128, 128], f32, kind="ExternalInput")
    # doubled constants: rows [0:50] chain A, rows [64:114] chain B
    expT2 = nc.dram_tensor("expT2", [128, K], f32, kind="ExternalInput")
    transr2 = nc.dram_tensor("transr2", [128, K], f32, kind="ExternalInput")
    cols2 = nc.dram_tensor("cols2", [128, 7], f32, kind="ExternalInput")
    # cols2 columns: 0=expstart 1=expend 2=startc 3=endc 4=bcol 5=iota 6=ones
    tagrow = nc.dram_tensor("tagrow", [1, BT], f32, kind="ExternalInput")
    onesr = nc.dram_tensor("onesr", [1, K], f32, kind="ExternalInput")
    out_d = nc.dram_tensor("out", [1, B_LOC], f32, kind="ExternalOutput")
    if dbg:
        dbg_e = nc.dram_tensor("dbg_e", [K, 13], f32, kind="ExternalOutput")
        dbg_ht = nc.dram_tensor("dbg_ht", [128, 16], f32, kind="ExternalOutput")
        dbg_gold = nc.dram_tensor("dbg_gold", [1, B_LOC], f32, kind="ExternalOutput")
        dbg_c = nc.dram_tensor("dbg_c", [1, B_LOC], f32, kind="ExternalOutput")
        dbg_lnz = nc.dram_tensor("dbg_lnz", [1, B_LOC], f32, kind="ExternalOutput")
        dbg_al1a = nc.dram_tensor("dbg_al1a", [K, BH], f32, kind="ExternalOutput")
        dbg_al1b = nc.dram_tensor("dbg_al1b", [K, BH], f32, kind="ExternalOutput")

    AF = mybir.ActivationFunctionType
    ALU = mybir.AluOpType
    AX = mybir.AxisListType

    with tile.TileContext(nc) as tc:
        with (
            tc.tile_pool(name="consts", bufs=1) as consts,
            tc.tile_pool(name="hnat", bufs=2) as hnat_pool,
            tc.tile_pool(name="ht", bufs=2) as ht_pool,
            tc.tile_pool(name="persist", bufs=1) as persist,
            tc.tile_pool(name="small", bufs=4) as small,
            tc.tile_pool(name="alpha", bufs=3) as alpha_pool,
            tc.tile_pool(name="tp_psum", bufs=2, space=bass.MemorySpace.PSUM) as tpsum,
            tc.tile_pool(name="big_psum", bufs=2, space=bass.MemorySpace.PSUM) as bpsum,
            tc.tile_pool(name="scan_psum", bufs=3, space=bass.MemorySpace.PSUM) as spsum,
        ):
            # ---- constants ----
            w_sb = consts.tile([128, D_CHUNKS, K], f32)
            nc.scalar.dma_start(w_sb[:], wq[:].rearrange("c p k -> p c k"))
            id_sb = consts.tile([128, 128], f32)
            nc.scalar.dma_start(id_sb[:], ident[:])
            expT_sb = consts.tile([128, K], f32)
            nc.scalar.dma_start(expT_sb[:], expT2[:])
            transr_sb = consts.tile([128, K], f32)
            nc.scalar.dma_start(transr_sb[:], transr2[:])
            cols_sb = consts.tile([128, 7], f32)
            nc.scalar.dma_start(cols_sb[:], cols2[:])
            tag_sb = consts.tile([1, BT], f32)
            nc.scalar.dma_start(tag_sb[:], tagrow[:])
            onesr_sb = consts.tile([1, K], f32)
            nc.scalar.dma_start(onesr_sb[:], onesr[:])

            expstart = cols_sb[:, 0:1]
            expend = cols_sb[:, 1:2]
            startc = cols_sb[:, 2:3]
            endc = cols_sb[:, 3:4]
            bcol = cols_sb[:, 4:5]
            iota = cols_sb[:, 5:6]
            onesc = cols_sb[:, 6:7]

            # persistent per-chain tensors; chain B lives at partitions 64:114
            E_a = persist.tile([K, BH, T], f32)
            E_bf = persist.tile([128, BH, T], f32)
            emis_a = persist.tile([K, BH, T], f32)
            emis_bf = persist.tile([128, BH, T], f32)
            oh_a = persist.tile([K, BH, T], f32)
            oh_bf = persist.tile([128, BH, T], f32)

            def half(c):
                """(row slice lo, chain tensors) for local sequence c."""
                if c < BH:
                    return 0, E_a, emis_a, oh_a, c
                return H2, E_bf, emis_bf, oh_bf, c - BH

            # ---- phase B: load + PE transpose + emissions GEMM ----
            for c in range(B_LOC):
                lo, E_t, em_t, _, a = half(c)
                hnat = hnat_pool.tile([128, 4, D], f32, tag="hnat")
                src = hid[c * T : (c + 1) * T, :].rearrange("(a p) d -> p a d", p=128)
                nc.sync.dma_start(hnat[:], src)

                ht = ht_pool.tile([128, D_CHUNKS, T], f32, tag="ht")
                for aa in range(4):
                    for dc in range(D_CHUNKS):
                        pst = tpsum.tile([128, 128], f32, tag="tp")
                        nc.tensor.transpose(
                            pst[:], hnat[:, aa, dc * 128 : (dc + 1) * 128], id_sb[:]
                        )
                        nc.vector.tensor_copy(
                            ht[:, dc, aa * 128 : (aa + 1) * 128], pst[:]
                        )

                if dbg and c == 0:
                    nc.sync.dma_start(dbg_ht[:], ht[:, 0, 0:16])
                ps = bpsum.tile([128, T], f32, tag="big")
                for dc in range(D_CHUNKS):
                    nc.tensor.matmul(
                        ps[lo : lo + K, :],
                        w_sb[:, dc, :],
                        ht[:, dc, :],
                        start=(dc == 0),
                        stop=(dc == D_CHUNKS - 1),
                    )
                nc.scalar.activation(
                    E_t[lo : lo + K, a, :], ps[lo : lo + K, :], AF.Exp,
                    bias=bcol[lo : lo + K],
                )
                nc.scalar.activation(
                    em_t[lo : lo + K, a, :], ps[lo : lo + K, :], AF.Identity,
                    bias=bcol[lo : lo + K],
                )

            if dbg:
                nc.sync.dma_start(dbg_e[:], E_a[0:K, 0, 0:13])
            # ---- phase C: gold score ----
            for c in range(B_LOC):
                lo, _, _, oh_t, a = half(c)
                psb = bpsum.tile([128, T], f32, tag="big")
                nc.tensor.matmul(
                    psb[lo : lo + K, :], onesr_sb[:],
                    tag_sb[:, c * T : (c + 1) * T], start=True, stop=True,
                )
                nc.vector.tensor_scalar(
                    oh_t[lo : lo + K, a, :], psb[lo : lo + K, :],
                    iota[lo : lo + K], None, ALU.is_equal,
                )
            for c in range(B_LOC):
                lo, _, em_t, oh_t, a = half(c)
                psc = bpsum.tile([128, T], f32, tag="big")
                nc.tensor.matmul(
                    psc[lo : lo + K, 0 : T - 1],
                    transr_sb[lo : lo + K, :],
                    oh_t[lo : lo + K, a, 0 : T - 1],
                    start=True, stop=True,
                )
                nc.vector.tensor_add(
                    em_t[lo : lo + K, a, 1:T],
                    em_t[lo : lo + K, a, 1:T],
                    psc[lo : lo + K, 0 : T - 1],
                )
            for lo, em_t, oh_t in ((0, emis_a, oh_a), (H2, emis_bf, oh_bf)):
                sl = slice(lo, lo + K)
                nc.vector.tensor_scalar_add(
                    em_t[sl, :, 0], em_t[sl, :, 0], startc[sl]
                )
                nc.vector.tensor_scalar_add(
                    em_t[sl, :, T - 1], em_t[sl, :, T - 1], endc[sl]
                )
                nc.vector.tensor_mul(oh_t[sl, :, :], oh_t[sl, :, :], em_t[sl, :, :])
            goldkb_a = persist.tile([K, BH], f32)
            goldkb_bf = persist.tile([128, BH], f32)
            nc.vector.tensor_reduce(goldkb_a[:], oh_a[0:K], AX.X, ALU.add)
            nc.vector.tensor_reduce(
                goldkb_bf[H2 : H2 + K], oh_bf[H2 : H2 + K], AX.X, ALU.add
            )
            gold_sb = small.tile([1, B_LOC], f32, tag="row")
            gps_a = bpsum.tile([1, BH], f32, tag="big")
            nc.tensor.matmul(
                gps_a[:], onesc[0:K], goldkb_a[:], start=True, stop=True
            )
            nc.scalar.copy(gold_sb[:, 0:BH], gps_a[:])
            gps_b = bpsum.tile([1, BH], f32, tag="big")
            nc.tensor.matmul(
                gps_b[:], onesc[H2 : H2 + K], goldkb_bf[H2 : H2 + K],
                start=True, stop=True,
            )
            nc.scalar.copy(gold_sb[:, BH:B_LOC], gps_b[:])
            if dbg:
                nc.sync.dma_start(dbg_gold[:], gold_sb[:])

            # ---- phase D: forward scan, two chains ----
            c_sb = persist.tile([1, B_LOC], f32)
            nc.vector.memset(c_sb[:], 0.0)
            for lo, E_t in ((0, E_a), (H2, E_bf)):
                sl = slice(lo, lo + K)
                nc.vector.tensor_scalar_mul(
                    E_t[sl, :, T - 1], E_t[sl, :, T - 1], expend[sl]
                )
            alpha_a = alpha_pool.tile([K, BH], f32, tag="aa")
            nc.vector.tensor_scalar_mul(alpha_a[:], E_a[0:K, :, 0], expstart[0:K])
            alpha_bf = alpha_pool.tile([128, BH], f32, tag="ab")
            slb = slice(H2, H2 + K)
            nc.vector.tensor_scalar_mul(
                alpha_bf[slb], E_bf[slb, :, 0], expstart[slb]
            )

            chains = [
                # (row-lo, E tile, alpha AP getter, alpha tag, C cols)
                [0, E_a, alpha_a[:], "aa", slice(0, BH)],
                [H2, E_bf, alpha_bf[slb], "ab", slice(BH, B_LOC)],
            ]

            for t in range(1, T):
                do_renorm = (t % RENORM == 0) and (t + 2 < T - 1)
                for ch in chains:
                    lo, E_t, alpha_ap, atag, ccols = ch
                    sl = slice(lo, lo + K)
                    ps = spsum.tile([128, BH], f32, tag="scan", name=f"ps{t}_{lo}")
                    nc.tensor.matmul(
                        ps[sl], expT_sb[sl], alpha_ap, start=True, stop=True
                    )
                    if do_renorm:
                        # side chain: s = sum(alpha_{t-1}); E[t+2] *= 1/s; C += ln s
                        sps = spsum.tile([1, BH], f32, tag="ssum", bufs=1, name=f"ss{t}_{lo}")
                        nc.tensor.matmul(
                            sps[:], onesc[sl], alpha_ap, start=True, stop=True
                        )
                        r_sb = small.tile([1, BH], f32, tag="row")
                        nc.vector.reciprocal(r_sb[:], sps[:])
                        psr = spsum.tile([128, BH], f32, tag="scan", name=f"pr{t}_{lo}")
                        nc.tensor.matmul(
                            psr[sl], onesr_sb[:], r_sb[:], start=True, stop=True
                        )
                        nc.vector.tensor_mul(
                            E_t[sl, :, t + 2], E_t[sl, :, t + 2], psr[sl]
                        )
                        lns = small.tile([1, BH], f32, tag="row")
                        nc.scalar.activation(lns[:], sps[:], AF.Ln)
                        nc.vector.tensor_add(
                            c_sb[:, ccols], c_sb[:, ccols], lns[:]
                        )
                    if lo == 0:
                        alpha_new = alpha_pool.tile([K, BH], f32, tag=atag)
                        new_ap = alpha_new[:]
                    else:
                        alpha_new = alpha_pool.tile([128, BH], f32, tag=atag)
                        new_ap = alpha_new[slb]
                    nc.vector.tensor_mul(new_ap, ps[sl], E_t[sl, :, t])
                    if dbg and t == 1:
                        nc.sync.dma_start(
                            dbg_al1a[:] if lo == 0 else dbg_al1b[:], new_ap
                        )
                    ch[2] = new_ap

            lnz = small.tile([1, B_LOC], f32, tag="row")
            for ch in chains:
                lo, E_t, alpha_ap, atag, ccols = ch
                sl = slice(lo, lo + K)
                zps = spsum.tile([1, BH], f32, tag="ssum", bufs=1, name=f"z{lo}")
                nc.tensor.matmul(zps[:], onesc[sl], alpha_ap, start=True, stop=True)
                nc.scalar.activation(lnz[:, ccols], zps[:], AF.Ln)
            if dbg:
                nc.sync.dma_start(dbg_c[:], c_sb[:])
                nc.sync.dma_start(dbg_lnz[:], lnz[:])
            nc.vector.tensor_add(lnz[:], lnz[:], c_sb[:])
            outrow = small.tile([1, B_LOC], f32, tag="row")
            nc.vector.tensor_sub(outrow[:], lnz[:], gold_sb[:])
            nc.sync.dma_start(out_d[:], outrow[:])

    nc.compile()
    return nc


def _get_compiled():
    if "nc" not in _COMPILED:
        _COMPILED["nc"] = _build()
    return _COMPILED["nc"]


def _doubled(col):
    """[50] -> [128] with copies at rows 0:50 and 64:114."""
    v = np.zeros(128, np.float32)
    v[0:K] = col
    v[H2 : H2 + K] = col
    return v


def kernel(full_hidden, tag_ids, mask, W, b, transitions, start_trans, end_trans):
    global LAST_RESULT
    from concourse.bass_utils import run_bass_kernel_spmd

    full_hidden = np.ascontiguousarray(np.asarray(full_hidden, dtype=np.float32))
    tags = np.asarray(tag_ids)
    W = np.asarray(W, dtype=np.float32)
    b = np.asarray(b, dtype=np.float32)
    transitions = np.asarray(transitions, dtype=np.float32)
    start_trans = np.asarray(start_trans, dtype=np.float32)
    end_trans = np.asarray(end_trans, dtype=np.float32)

    nc = _get_compiled()

    expT2 = np.zeros((128, K), np.float32)
    expT2[0:K] = np.exp(transitions)
    expT2[H2 : H2 + K] = np.exp(transitions)
    transr2 = np.zeros((128, K), np.float32)
    transr2[0:K] = transitions
    transr2[H2 : H2 + K] = transitions
    cols2 = np.stack(
        [
            _doubled(np.exp(start_trans)),
            _doubled(np.exp(end_trans)),
            _doubled(start_trans),
            _doubled(end_trans),
            _doubled(b),
            _doubled(np.arange(K, dtype=np.float32)),
            _doubled(np.ones(K, np.float32)),
        ],
        axis=1,
    ).astype(np.float32)

    common = {
        "wq": np.ascontiguousarray(W.reshape(D_CHUNKS, 128, K)),
        "ident": np.eye(128, dtype=np.float32),
        "expT2": expT2,
        "transr2": transr2,
        "cols2": np.ascontiguousarray(cols2),
        "onesr": np.ones((1, K), np.float32),
    }
    in_maps = []
    for c in range(N_CORES):
        sl = slice(c * B_LOC, (c + 1) * B_LOC)
        in_maps.append(
            {
                "hid": np.ascontiguousarray(full_hidden[sl].reshape(BT, D)),
                "tagrow": np.ascontiguousarray(
                    tags[sl].astype(np.float32).reshape(1, BT)
                ),
                **common,
            }
        )

    res = run_bass_kernel_spmd(nc, in_maps, core_ids=list(range(N_CORES)))
    LAST_RESULT = res
    out = np.concatenate(
        [np.asarray(res.results[c]["out"]).reshape(B_LOC) for c in range(N_CORES)]
    )
    return out.astype(np.float32)



# revision 8
# speedup vs baseline: 3.4935x; 3.4935x over previous
"""CRF negative-log-likelihood kernel for Trainium2, SPMD over 8 NeuronCores.

Strategy (v2)
-------------
Data-parallel over batch: core c handles sequences b in [c*8, (c+1)*8).

Per core (B=8 local sequences, T=512, K=50 tags, D=1024):

1. Emissions GEMM in bf16 from HOST-pre-transposed hidden (hidT packed
   [p, seq, dchunk, t] so DMA lines are 8KB-contiguous and no on-device
   transpose is needed).  Per sequence: 8 accumulating matmuls
   [128 x 50 x 512] -> PSUM emis [50, 512].
2. E' build (renorm-free scan): E_raw = exp(emis + b) (ScalarE, bf16);
   cs = s0 * colsum(E_raw) via ones-matmul; E' = E_raw * (1/cs)
   (reciprocal broadcast over partitions with a rank-1 matmul).  The
   per-column log corrections ln(cs) accumulate off the critical path:
   log_Z = ln(w . a) + sum_t ln(cs_t).  With s0 = mean(exp(transitions))
   the scaled recurrence drifts only O(sqrt(T) * 0.02) e-folds: no
   renormalization needed inside the scan at all.
3. Partition function with HALVED serial depth: split the matrix-product
   chain in the middle,
       log_Z = ln( w . a ),
       a = A_255 ... A_1 alpha_0          (forward chain,  255 steps)
       w = A_256^T ... A_511^T exp(end)   (backward chain, 256 steps)
   where A_t = diag(E'_t) M^T.  Forward step: PE matmul (M as lhsT) then
   DVE multiply by E'_t.  Backward step: DVE multiply by E'_t then PE
   matmul (M^T as lhsT).  The two chains are independent and ping-pong
   PE<->DVE concurrently; everything is bf16 single-pass on the PE.
4. Gold score: emission part on device via ONE scalar_tensor_tensor per
   sequence: out = (bcast(tags) == iota) * emis with accum_out giving
   the per-tag sums; a ones-matmul reduces over tags.  The transition +
   start/end part is a pure function of tag_ids, computed on host.
"""

import numpy as np

B_FULL = 64
B_LOC = 8
T = 512
K = 50
D = 1024
DC = 8  # d chunks of 128
N_CORES = 8
BT = B_LOC * T  # 4096
MID = 256  # fwd handles t=1..255, bwd t=511..256

_COMPILED = {}
LAST_RESULT = None


def _build():
    import concourse.bass as bass
    import concourse.tile as tile
    from concourse import bacc, mybir

    f32 = mybir.dt.float32
    bf16 = mybir.dt.bfloat16

    nc = bacc.Bacc(
        "TRN2",
        target_bir_lowering=False,
        debug=False,
        num_devices=N_CORES,
    )

    hidT = nc.dram_tensor("hidT", [128, B_LOC, DC, T], bf16, kind="ExternalInput")
    wq = nc.dram_tensor("wq", [128, DC, K], bf16, kind="ExternalInput")
    mf = nc.dram_tensor("mf", [K, K], bf16, kind="ExternalInput")
    mb = nc.dram_tensor("mb", [K, K], bf16, kind="ExternalInput")
    tagr = nc.dram_tensor("tagr", [1, BT], bf16, kind="ExternalInput")
    winit = nc.dram_tensor("winit", [K, B_LOC], bf16, kind="ExternalInput")
    colsA = nc.dram_tensor("colsA", [K, 4], f32, kind="ExternalInput")
    # colsA columns: 0=b bias, 1=exp(start), 2=iota, 3=ones(f32)
    onesb = nc.dram_tensor("onesb", [K, 2], bf16, kind="ExternalInput")
    # onesb columns: 0=s0 (colsum stationary), 1=ones (dot stationary)
    onesrow = nc.dram_tensor("onesrow", [1, K], bf16, kind="ExternalInput")
    out_d = nc.dram_tensor("out", [1, B_LOC], f32, kind="ExternalOutput")

    AF = mybir.ActivationFunctionType
    ALU = mybir.AluOpType
    AX = mybir.AxisListType

    with tile.TileContext(nc) as tc:
        with (
            tc.tile_pool(name="consts", bufs=1) as consts,
            tc.tile_pool(name="persist", bufs=1) as persist,
            tc.tile_pool(name="small", bufs=4) as small,
            tc.tile_pool(name="alpha", bufs=3) as apool,
            tc.tile_pool(name="xb", bufs=3) as xpool,
        ):
            # ---- constants ----
            w_sb = consts.tile([128, DC, K], bf16)
            nc.scalar.dma_start(w_sb[:], wq[:])
            mf_sb = consts.tile([K, K], bf16)
            nc.scalar.dma_start(mf_sb[:], mf[:])
            mb_sb = consts.tile([K, K], bf16)
            nc.scalar.dma_start(mb_sb[:], mb[:])
            tag_sb = consts.tile([1, BT], bf16)
            nc.scalar.dma_start(tag_sb[:], tagr[:])
            winit_sb = consts.tile([K, B_LOC], bf16)
            nc.scalar.dma_start(winit_sb[:], winit[:])
            colsA_sb = consts.tile([K, 4], f32)
            nc.scalar.dma_start(colsA_sb[:], colsA[:])
            onesb_sb = consts.tile([K, 2], bf16)
            nc.scalar.dma_start(onesb_sb[:], onesb[:])
            onesrow_sb = consts.tile([1, K], bf16)
            nc.scalar.dma_start(onesrow_sb[:], onesrow[:])

            bcol = colsA_sb[:, 0:1]
            expstart = colsA_sb[:, 1:2]
            iota = colsA_sb[:, 2:3]
            onesf = colsA_sb[:, 3:4]
            s0col = onesb_sb[:, 0:1]
            onescol = onesb_sb[:, 1:2]

            # ---- persistent tensors ----
            hid_sb = persist.tile([128, B_LOC, DC, T], bf16)
            E2 = persist.tile([K, B_LOC, T], bf16)  # scaled E'
            Eraw = persist.tile([K, B_LOC, T], bf16)
            emis = persist.tile([K, B_LOC, T], bf16)
            lncs = persist.tile([1, B_LOC, T], f32)
            lnsums = persist.tile([1, B_LOC], f32)
            goldk = persist.tile([K, B_LOC], f32)
            scr = persist.tile([K, T], bf16)  # scatter target for stt

            # ---- per-sequence prep: DMA, GEMM, E', gold ----
            with (
                tc.tile_pool(name="pe_ps", bufs=2, space=bass.MemorySpace.PSUM) as pe_ps,
                tc.tile_pool(name="cs_ps", bufs=2, space=bass.MemorySpace.PSUM) as cs_ps,
                tc.tile_pool(name="bc_ps", bufs=2, space=bass.MemorySpace.PSUM) as bc_ps,
            ):
              for s in range(B_LOC):
                nc.sync.dma_start(hid_sb[:, s, :, :], hidT[:, s, :, :])

                ps_e = pe_ps.tile([K, T], f32, tag="pse")
                for dc in range(DC):
                    nc.tensor.matmul(
                        ps_e[:],
                        w_sb[:, dc, :],
                        hid_sb[:, s, dc, :],
                        start=(dc == 0),
                        stop=(dc == DC - 1),
                    )
                # E_raw = exp(emis + b); emis copy for the gold score
                nc.scalar.activation(Eraw[:, s, :], ps_e[:], AF.Exp, bias=bcol)
                nc.scalar.activation(emis[:, s, :], ps_e[:], AF.Identity, bias=bcol)
                # cs = s0 * colsum(E_raw)
                ps_cs = cs_ps.tile([1, T], f32, tag="cs")
                nc.tensor.matmul(ps_cs[:], s0col, Eraw[:, s, :], start=True, stop=True)
                nc.scalar.activation(lncs[:, s, :], ps_cs[:], AF.Ln)
                rcs = small.tile([1, T], bf16, tag="rcs")
                with nc.allow_low_precision(reason="ln(cs) correction absorbs recip rounding"):
                    nc.vector.reciprocal(rcs[:], ps_cs[:])
                # per-seq partial sum of ln(cs) (keeps the big reduce off the tail)
                nc.vector.tensor_reduce(lnsums[:, s : s + 1], lncs[:, s, :], AX.X, ALU.add)
                ps_bc = bc_ps.tile([K, T], f32, tag="bc")
                nc.tensor.matmul(ps_bc[:], onesrow_sb[:], rcs[:], start=True, stop=True)
                nc.vector.tensor_mul(E2[:, s, :], Eraw[:, s, :], ps_bc[:])
                # gold emissions: bcast tags, compare to iota, dot with emis
                ps_t = bc_ps.tile([K, T], f32, tag="bc")
                nc.tensor.matmul(
                    ps_t[:], onesrow_sb[:], tag_sb[:, s * T : (s + 1) * T],
                    start=True, stop=True,
                )
                nc.vector.scalar_tensor_tensor(
                    scr[:],
                    ps_t[:],
                    iota,
                    emis[:, s, :],
                    ALU.is_equal,
                    ALU.mult,
                    accum_out=goldk[:, s : s + 1],
                )

            # ---- forward/backward scan ----
            with (
                tc.tile_pool(name="sf_ps", bufs=3, space=bass.MemorySpace.PSUM) as sf_ps,
                tc.tile_pool(name="sb_ps", bufs=3, space=bass.MemorySpace.PSUM) as sb_ps,
                tc.tile_pool(name="z_ps", bufs=2, space=bass.MemorySpace.PSUM) as z_ps,
            ):
              alpha = apool.tile([K, B_LOC], bf16, tag="a")
              nc.vector.tensor_scalar_mul(alpha[:], E2[:, :, 0], expstart)
              alpha_ap = alpha[:]
              w_ap = winit_sb[:]

              for i in range(1, MID):
                tf = i
                tb = T - i
                ps_f = sf_ps.tile([K, B_LOC], f32, tag="psf", name=f"pf{i}")
                nc.tensor.matmul(ps_f[:], mf_sb[:], alpha_ap, start=True, stop=True)
                x_b = xpool.tile([K, B_LOC], bf16, tag="x", name=f"xb{i}")
                nc.vector.tensor_mul(x_b[:], w_ap, E2[:, :, tb])
                ps_b = sb_ps.tile([K, B_LOC], f32, tag="psb", name=f"pb{i}")
                nc.tensor.matmul(ps_b[:], mb_sb[:], x_b[:], start=True, stop=True)
                alpha_new = apool.tile([K, B_LOC], bf16, tag="a", name=f"al{i}")
                nc.vector.tensor_mul(alpha_new[:], ps_f[:], E2[:, :, tf])
                alpha_ap = alpha_new[:]
                w_ap = ps_b[:]

              # tail: bwd needs one more step (t = MID)
              x_l = xpool.tile([K, B_LOC], bf16, tag="x", name="xlast")
              nc.vector.tensor_mul(x_l[:], w_ap, E2[:, :, MID])
              ps_l = sb_ps.tile([K, B_LOC], f32, tag="psb", name="pblast")
              nc.tensor.matmul(ps_l[:], mb_sb[:], x_l[:], start=True, stop=True)

              # ---- epilogue: log_Z = ln(w . a) + sum ln(cs); out = log_Z - goldE
              wdot = small.tile([K, B_LOC], bf16, tag="wdot")
              nc.vector.tensor_mul(wdot[:], ps_l[:], alpha_ap)
              ps_z = z_ps.tile([1, B_LOC], f32, tag="z")
              nc.tensor.matmul(ps_z[:], onescol, wdot[:], start=True, stop=True)
              lnz = small.tile([1, B_LOC], f32, tag="row")
              nc.scalar.activation(lnz[:], ps_z[:], AF.Ln)
              ps_g = z_ps.tile([1, B_LOC], f32, tag="z")
              nc.tensor.matmul(ps_g[:], onesf, goldk[:], start=True, stop=True)
              acc = small.tile([1, B_LOC], f32, tag="row")
              nc.vector.tensor_add(acc[:], lnz[:], lnsums[:])
              outrow = small.tile([1, B_LOC], f32, tag="row")
              nc.vector.tensor_sub(outrow[:], acc[:], ps_g[:])
              nc.sync.dma_start(out_d[:], outrow[:])

    nc.compile()
    return nc


def _get_compiled():
    if "nc" not in _COMPILED:
        _COMPILED["nc"] = _build()
    return _COMPILED["nc"]


def _host_inputs(full_hidden, tag_ids, W, b, transitions, start_trans, end_trans):
    """Build the per-core in_maps plus host-side gold transition scores."""
    import ml_dtypes

    bf16 = ml_dtypes.bfloat16

    full_hidden = np.asarray(full_hidden, dtype=np.float32)
    tags = np.asarray(tag_ids).astype(np.int64)
    W = np.asarray(W, dtype=np.float32)
    b = np.asarray(b, dtype=np.float32)
    transitions = np.asarray(transitions, dtype=np.float32)
    start_trans = np.asarray(start_trans, dtype=np.float32)
    end_trans = np.asarray(end_trans, dtype=np.float32)

    M = np.exp(transitions)
    s0 = float(M.mean())

    common = {
        "wq": np.ascontiguousarray(
            W.reshape(DC, 128, K).transpose(1, 0, 2)
        ).astype(bf16),
        "mf": M.astype(bf16),
        "mb": np.ascontiguousarray(M.T).astype(bf16),
        "winit": np.tile(
            np.exp(end_trans)[:, None].astype(np.float32), (1, B_LOC)
        ).astype(bf16),
        "colsA": np.ascontiguousarray(
            np.stack(
                [b, np.exp(start_trans), np.arange(K, dtype=np.float32),
                 np.ones(K, np.float32)],
                axis=1,
            )
        ),
        "onesb": np.ascontiguousarray(
            np.stack(
                [np.full(K, s0, np.float32), np.ones(K, np.float32)], axis=1
            )
        ).astype(bf16),
        "onesrow": np.ones((1, K), np.float32).astype(bf16),
    }

    in_maps = []
    for c in range(N_CORES):
        sl = slice(c * B_LOC, (c + 1) * B_LOC)
        h = full_hidden[sl]  # [8, 512, 1024]
        hidT = np.ascontiguousarray(
            h.reshape(B_LOC, T, DC, 128).transpose(3, 0, 2, 1)
        ).astype(bf16)  # [128, seq, dc, t]
        in_maps.append(
            {
                "hidT": hidT,
                "tagr": tags[sl].astype(np.float32).reshape(1, BT).astype(bf16),
                **common,
            }
        )

    # host part of the gold score: transitions + start/end (tags only)
    gold_trans = (
        transitions[tags[:, :-1], tags[:, 1:]].sum(axis=1)
        + start_trans[tags[:, 0]]
        + end_trans[tags[:, -1]]
    ).astype(np.float32)
    return in_maps, gold_trans


def kernel(full_hidden, tag_ids, mask, W, b, transitions, start_trans, end_trans):
    global LAST_RESULT
    from concourse.bass_utils import run_bass_kernel_spmd

    in_maps, gold_trans = _host_inputs(
        full_hidden, tag_ids, W, b, transitions, start_trans, end_trans
    )
    nc = _get_compiled()
    res = run_bass_kernel_spmd(nc, in_maps, core_ids=list(range(N_CORES)))
    LAST_RESULT = res
    dev = np.concatenate(
        [np.asarray(res.results[c]["out"]).reshape(B_LOC) for c in range(N_CORES)]
    ).astype(np.float32)
    return dev - gold_trans


# revision 10
# speedup vs baseline: 3.5433x; 1.0143x over previous
"""CRF negative-log-likelihood kernel for Trainium2, SPMD over 8 NeuronCores.

Strategy (v2)
-------------
Data-parallel over batch: core c handles sequences b in [c*8, (c+1)*8).

Per core (B=8 local sequences, T=512, K=50 tags, D=1024):

1. Emissions GEMM in bf16 from HOST-pre-transposed hidden (hidT packed
   [p, seq, dchunk, t] so DMA lines are 8KB-contiguous and no on-device
   transpose is needed).  Per sequence: 8 accumulating matmuls
   [128 x 50 x 512] -> PSUM emis [50, 512].
2. E' build (renorm-free scan): E_raw = exp(emis + b) (ScalarE, bf16);
   cs = s0 * colsum(E_raw) via ones-matmul; E' = E_raw * (1/cs)
   (reciprocal broadcast over partitions with a rank-1 matmul).  The
   per-column log corrections ln(cs) accumulate off the critical path:
   log_Z = ln(w . a) + sum_t ln(cs_t).  With s0 = mean(exp(transitions))
   the scaled recurrence drifts only O(sqrt(T) * 0.02) e-folds: no
   renormalization needed inside the scan at all.
3. Partition function with HALVED serial depth: split the matrix-product
   chain in the middle,
       log_Z = ln( w . a ),
       a = A_255 ... A_1 alpha_0          (forward chain,  255 steps)
       w = A_256^T ... A_511^T exp(end)   (backward chain, 256 steps)
   where A_t = diag(E'_t) M^T.  Forward step: PE matmul (M as lhsT) then
   DVE multiply by E'_t.  Backward step: DVE multiply by E'_t then PE
   matmul (M^T as lhsT).  The two chains are independent and ping-pong
   PE<->DVE concurrently; everything is bf16 single-pass on the PE.
4. Gold score: emission part on device via ONE scalar_tensor_tensor per
   sequence: out = (bcast(tags) == iota) * emis with accum_out giving
   the per-tag sums; a ones-matmul reduces over tags.  The transition +
   start/end part is a pure function of tag_ids, computed on host.
"""

import numpy as np

B_FULL = 64
B_LOC = 8
T = 512
K = 50
D = 1024
DC = 8  # d chunks of 128
N_CORES = 8
BT = B_LOC * T  # 4096
MID = 256  # fwd handles t=1..255, bwd t=511..256

_COMPILED = {}
LAST_RESULT = None


def _build():
    import concourse.bass as bass
    import concourse.tile as tile
    from concourse import bacc, mybir

    f32 = mybir.dt.float32
    bf16 = mybir.dt.bfloat16

    nc = bacc.Bacc(
        "TRN2",
        target_bir_lowering=False,
        debug=False,
        num_devices=N_CORES,
    )

    hidT = nc.dram_tensor("hidT", [128, B_LOC, DC, T], bf16, kind="ExternalInput")
    wq = nc.dram_tensor("wq", [128, DC, K], bf16, kind="ExternalInput")
    mf = nc.dram_tensor("mf", [K, K], bf16, kind="ExternalInput")
    mb = nc.dram_tensor("mb", [K, K], bf16, kind="ExternalInput")
    tagr = nc.dram_tensor("tagr", [1, BT], bf16, kind="ExternalInput")
    winit = nc.dram_tensor("winit", [K, B_LOC], bf16, kind="ExternalInput")
    colsA = nc.dram_tensor("colsA", [K, 4], f32, kind="ExternalInput")
    # colsA columns: 0=b bias, 1=exp(start), 2=iota, 3=ones(f32)
    onesb = nc.dram_tensor("onesb", [K, 2], bf16, kind="ExternalInput")
    # onesb columns: 0=s0 (colsum stationary), 1=ones (dot stationary)
    onesrow = nc.dram_tensor("onesrow", [1, K], bf16, kind="ExternalInput")
    out_d = nc.dram_tensor("out", [1, B_LOC], f32, kind="ExternalOutput")

    AF = mybir.ActivationFunctionType
    ALU = mybir.AluOpType
    AX = mybir.AxisListType

    with tile.TileContext(nc) as tc:
        with (
            tc.tile_pool(name="consts", bufs=1) as consts,
            tc.tile_pool(name="persist", bufs=1) as persist,
            tc.tile_pool(name="small", bufs=4) as small,
            tc.tile_pool(name="alpha", bufs=4) as apool,
            tc.tile_pool(name="xb", bufs=4) as xpool,
        ):
            # ---- constants ----
            w_sb = consts.tile([128, DC, K], bf16)
            nc.scalar.dma_start(w_sb[:], wq[:])
            mf_sb = consts.tile([K, K], bf16)
            nc.scalar.dma_start(mf_sb[:], mf[:])
            mb_sb = consts.tile([K, K], bf16)
            nc.scalar.dma_start(mb_sb[:], mb[:])
            tag_sb = consts.tile([1, BT], bf16)
            nc.scalar.dma_start(tag_sb[:], tagr[:])
            winit_sb = consts.tile([K, B_LOC], bf16)
            nc.scalar.dma_start(winit_sb[:], winit[:])
            colsA_sb = consts.tile([K, 4], f32)
            nc.scalar.dma_start(colsA_sb[:], colsA[:])
            onesb_sb = consts.tile([K, 2], bf16)
            nc.scalar.dma_start(onesb_sb[:], onesb[:])
            onesrow_sb = consts.tile([1, K], bf16)
            nc.scalar.dma_start(onesrow_sb[:], onesrow[:])

            bcol = colsA_sb[:, 0:1]
            expstart = colsA_sb[:, 1:2]
            iota = colsA_sb[:, 2:3]
            onesf = colsA_sb[:, 3:4]
            s0col = onesb_sb[:, 0:1]
            onescol = onesb_sb[:, 1:2]

            # ---- persistent tensors ----
            hid_sb = persist.tile([128, B_LOC, DC, T], bf16)
            E2 = persist.tile([K, B_LOC, T], bf16)  # scaled E'
            Eraw = persist.tile([K, B_LOC, T], bf16)
            emis = persist.tile([K, B_LOC, T], bf16)
            lncs = persist.tile([1, B_LOC, T], f32)
            lnsums = persist.tile([1, B_LOC], f32)
            goldk = persist.tile([K, B_LOC], f32)
            scr = persist.tile([K, T], bf16)  # scatter target for stt

            # ---- per-sequence prep: DMA, GEMM, E', gold ----
            with (
                tc.tile_pool(name="pe_ps", bufs=2, space=bass.MemorySpace.PSUM) as pe_ps,
                tc.tile_pool(name="cs_ps", bufs=2, space=bass.MemorySpace.PSUM) as cs_ps,
                tc.tile_pool(name="bc_ps", bufs=2, space=bass.MemorySpace.PSUM) as bc_ps,
            ):
              for s in range(B_LOC):
                nc.sync.dma_start(hid_sb[:, s, :, :], hidT[:, s, :, :])

                ps_e = pe_ps.tile([K, T], f32, tag="pse")
                for dc in range(DC):
                    nc.tensor.matmul(
                        ps_e[:],
                        w_sb[:, dc, :],
                        hid_sb[:, s, dc, :],
                        start=(dc == 0),
                        stop=(dc == DC - 1),
                    )
                # E_raw = exp(emis + b); emis copy for the gold score
                nc.scalar.activation(Eraw[:, s, :], ps_e[:], AF.Exp, bias=bcol)
                nc.scalar.activation(emis[:, s, :], ps_e[:], AF.Identity, bias=bcol)
                # cs = s0 * colsum(E_raw)
                ps_cs = cs_ps.tile([1, T], f32, tag="cs")
                nc.tensor.matmul(ps_cs[:], s0col, Eraw[:, s, :], start=True, stop=True)
                # ln(cs) with free-dim sum fused; scale 1/cs = exp(-ln cs) on
                # ScalarE (DVE reciprocal is ~3.3us per row - way too slow)
                nc.scalar.activation(
                    lncs[:, s, :], ps_cs[:], AF.Ln,
                    accum_out=lnsums[:, s : s + 1],
                )
                rcs = small.tile([1, T], bf16, tag="rcs")
                nc.scalar.activation(rcs[:], lncs[:, s, :], AF.Exp, scale=-1.0)
                ps_bc = bc_ps.tile([K, T], f32, tag="bc")
                nc.tensor.matmul(ps_bc[:], onesrow_sb[:], rcs[:], start=True, stop=True)
                nc.vector.tensor_mul(E2[:, s, :], Eraw[:, s, :], ps_bc[:])
                # gold emissions: bcast tags, compare to iota, dot with emis
                ps_t = bc_ps.tile([K, T], f32, tag="bc")
                nc.tensor.matmul(
                    ps_t[:], onesrow_sb[:], tag_sb[:, s * T : (s + 1) * T],
                    start=True, stop=True,
                )
                nc.vector.scalar_tensor_tensor(
                    scr[:],
                    ps_t[:],
                    iota,
                    emis[:, s, :],
                    ALU.is_equal,
                    ALU.mult,
                    accum_out=goldk[:, s : s + 1],
                )

            # ---- forward/backward scan ----
            with (
                tc.tile_pool(name="sf_ps", bufs=3, space=bass.MemorySpace.PSUM) as sf_ps,
                tc.tile_pool(name="sb_ps", bufs=3, space=bass.MemorySpace.PSUM) as sb_ps,
                tc.tile_pool(name="z_ps", bufs=2, space=bass.MemorySpace.PSUM) as z_ps,
            ):
              alpha = apool.tile([K, B_LOC], bf16, tag="a")
              nc.vector.tensor_scalar_mul(alpha[:], E2[:, :, 0], expstart)
              alpha_ap = alpha[:]
              w_ap = winit_sb[:]

              for i in range(1, MID):
                tf = i
                tb = T - i
                ps_f = sf_ps.tile([K, B_LOC], f32, tag="psf", name=f"pf{i}")
                nc.tensor.matmul(ps_f[:], mf_sb[:], alpha_ap, start=True, stop=True)
                x_b = xpool.tile([K, B_LOC], bf16, tag="x", name=f"xb{i}")
                nc.vector.tensor_mul(x_b[:], w_ap, E2[:, :, tb])
                ps_b = sb_ps.tile([K, B_LOC], f32, tag="psb", name=f"pb{i}")
                nc.tensor.matmul(ps_b[:], mb_sb[:], x_b[:], start=True, stop=True)
                alpha_new = apool.tile([K, B_LOC], bf16, tag="a", name=f"al{i}")
                nc.vector.tensor_mul(alpha_new[:], ps_f[:], E2[:, :, tf])
                alpha_ap = alpha_new[:]
                w_ap = ps_b[:]

              # tail: bwd needs one more step (t = MID)
              x_l = xpool.tile([K, B_LOC], bf16, tag="x", name="xlast")
              nc.vector.tensor_mul(x_l[:], w_ap, E2[:, :, MID])
              ps_l = sb_ps.tile([K, B_LOC], f32, tag="psb", name="pblast")
              nc.tensor.matmul(ps_l[:], mb_sb[:], x_l[:], start=True, stop=True)

              # ---- epilogue: log_Z = ln(w . a) + sum ln(cs); out = log_Z - goldE
              wdot = small.tile([K, B_LOC], bf16, tag="wdot")
              nc.vector.tensor_mul(wdot[:], ps_l[:], alpha_ap)
              ps_z = z_ps.tile([1, B_LOC], f32, tag="z")
              nc.tensor.matmul(ps_z[:], onescol, wdot[:], start=True, stop=True)
              lnz = small.tile([1, B_LOC], f32, tag="row")
              nc.scalar.activation(lnz[:], ps_z[:], AF.Ln)
              ps_g = z_ps.tile([1, B_LOC], f32, tag="z")
              nc.tensor.matmul(ps_g[:], onesf, goldk[:], start=True, stop=True)
              acc = small.tile([1, B_LOC], f32, tag="row")
              nc.vector.tensor_add(acc[:], lnz[:], lnsums[:])
              outrow = small.tile([1, B_LOC], f32, tag="row")
              nc.vector.tensor_sub(outrow[:], acc[:], ps_g[:])
              nc.sync.dma_start(out_d[:], outrow[:])

    nc.compile()
    return nc


def _get_compiled():
    if "nc" not in _COMPILED:
        _COMPILED["nc"] = _build()
    return _COMPILED["nc"]


def _host_inputs(full_hidden, tag_ids, W, b, transitions, start_trans, end_trans):
    """Build the per-core in_maps plus host-side gold transition scores."""
    import ml_dtypes

    bf16 = ml_dtypes.bfloat16

    full_hidden = np.asarray(full_hidden, dtype=np.float32)
    tags = np.asarray(tag_ids).astype(np.int64)
    W = np.asarray(W, dtype=np.float32)
    b = np.asarray(b, dtype=np.float32)
    transitions = np.asarray(transitions, dtype=np.float32)
    start_trans = np.asarray(start_trans, dtype=np.float32)
    end_trans = np.asarray(end_trans, dtype=np.float32)

    M = np.exp(transitions)
    s0 = float(M.mean())

    common = {
        "wq": np.ascontiguousarray(
            W.reshape(DC, 128, K).transpose(1, 0, 2)
        ).astype(bf16),
        "mf": M.astype(bf16),
        "mb": np.ascontiguousarray(M.T).astype(bf16),
        "winit": np.tile(
            np.exp(end_trans)[:, None].astype(np.float32), (1, B_LOC)
        ).astype(bf16),
        "colsA": np.ascontiguousarray(
            np.stack(
                [b, np.exp(start_trans), np.arange(K, dtype=np.float32),
                 np.ones(K, np.float32)],
                axis=1,
            )
        ),
        "onesb": np.ascontiguousarray(
            np.stack(
                [np.full(K, s0, np.float32), np.ones(K, np.float32)], axis=1
            )
        ).astype(bf16),
        "onesrow": np.ones((1, K), np.float32).astype(bf16),
    }

    in_maps = []
    for c in range(N_CORES):
        sl = slice(c * B_LOC, (c + 1) * B_LOC)
        h = full_hidden[sl]  # [8, 512, 1024]
        hidT = np.ascontiguousarray(
            h.reshape(B_LOC, T, DC, 128).transpose(3, 0, 2, 1)
        ).astype(bf16)  # [128, seq, dc, t]
        in_maps.append(
            {
                "hidT": hidT,
                "tagr": tags[sl].astype(np.float32).reshape(1, BT).astype(bf16),
                **common,
            }
        )

    # host part of the gold score: transitions + start/end (tags only)
    gold_trans = (
        transitions[tags[:, :-1], tags[:, 1:]].sum(axis=1)
        + start_trans[tags[:, 0]]
        + end_trans[tags[:, -1]]
    ).astype(np.float32)
    return in_maps, gold_trans


def kernel(full_hidden, tag_ids, mask, W, b, transitions, start_trans, end_trans):
    global LAST_RESULT
    from concourse.bass_utils import run_bass_kernel_spmd

    in_maps, gold_trans = _host_inputs(
        full_hidden, tag_ids, W, b, transitions, start_trans, end_trans
    )
    nc = _get_compiled()
    res = run_bass_kernel_spmd(nc, in_maps, core_ids=list(range(N_CORES)))
    LAST_RESULT = res
    dev = np.concatenate(
        [np.asarray(res.results[c]["out"]).reshape(B_LOC) for c in range(N_CORES)]
    ).astype(np.float32)
    return dev - gold_trans
